# revision 36
# baseline (speedup 1.0000x reference)
"""Trainium2 Bass kernel for nn_DecoderLayer_68212670595779.

Head-sharded attention (4 heads/core x batch over 8 cores), exchanged with
two half-size 8-rank AllToAlls (one per local head pair, overlapped with
attention / split Wo), token-quarter-parallel FFN. Attention uses q-major
AV accumulation ([q, 64+denominator] PSUM tiles, region-major accumulation
groups — interleaved groups sharing a PSUM bank corrupt all but the last
region on this hardware) with per-partition-scalar softmax normalization
and a PE transpose back to head-dim-major before staging.

Scheduling (measured 329-347 us/layer across runs on the repeat-loop
harness, ~+/-9 us run noise, rel err 5.878e-3; baseline was ~345 us):
- QKV projections interleave with head-pair-0 attention per token quarter,
  and each block's first LAG score/exp units are emitted before the
  previous block's AV pass, so the scalar engine always has exp work
  queued while the PE runs AV/projections.
- W1's first half is prefetched during attention into the SBUF slot that
  held xT (same shape; SP-ring chunk DMAs with a WAR dep on the last xT
  reader), removing the phase-1 -> phase-2 weight-load stall.  The
  RESIDENT chunks 0-3 compute first (chunk 4-7 DMAs queue behind the
  collective stub on the SP ring; issuing them upfront overlaps the
  fg0-3 matmuls).
- In the real build, A2A readbacks ride the gpsimd SWDGE ring (ordered
  behind the collective; waits don't block SP staging/weight DMAs);
  recombine adds run on gpsimd.  In For_i timing builds SWDGE DMAs don't
  compile, so readbacks and the collective-stub copy use the SP ring
  (an ACT-ring transfer would stall the exp stream: hwdge transfers
  occupy their issuing engine for the transfer duration, measured).
- The kc0-3 half of Wo runs inside the head-pair-1 attention window
  (spare PSUM banks, partial parked in SBUF with bo folded in); only the
  kc4-7 half remains on the post-A2A#2 critical path.
- Residual accumulates in place in attn_sb; per-t4 W2 chains keep the
  four LayerNorms staggered.

Known HW cost model (microbenchmarked): every stationary change costs
~66ns unhidden LDWEIGHTS (no FWL in this toolchain; --enable-ldw-opt is
incompatible with bass Ldweights).  Chained 65-col AV matmuls: ~78ns.
Same-stationary consecutive MMs skip the reload (~43ns) but the tile
scheduler interleaves other matmuls between them, so a t-major AV
restructure measured SLOWER end-to-end (369 us) — kept r4-major chains.
Col-tiled AV pairs (64-col stationary, N=512) hit full peak (207ns/pair)
but need a separate denominator pass + per-column normalize; net ~even.
"""
import sys

sys.path.insert(0, "/opt/trn_rl_repo")

import numpy as np
import ml_dtypes
from contextlib import ExitStack

import concourse.bass as bass
import concourse.mybir as mybir
import concourse.tile as tile
from concourse.vector_clock import ScopedClock
from concourse.bass_utils import run_bass_kernel_spmd

BF16 = ml_dtypes.bfloat16
FP32 = mybir.dt.float32
BF = mybir.dt.bfloat16
AF = mybir.ActivationFunctionType
ALU = mybir.AluOpType
AX = mybir.AxisListType

B, S, D, H, HD, FF, P = 2, 2048, 1024, 16, 64, 4096, 128
NCORES = 8


# ---------------------------------------------------------------------------
# Workaround: this walrus build allows only ONE semaphore wait on a CTRL
# (Drain) instruction; TileContext's final drain carries one wait per busy
# proc. Split the waits across a chain of drains on the same engine.
def _patched_drain_and_barrier(self, tick_clock, wait_clock):
    nc = self.nc
    drain_inst = nc.sync.drain()
    wait_clock.add_sem_waits(
        drain_inst.ins, ScopedClock({None: tick_clock.global_clock})
    )
    si = drain_inst.ins.sync_info
    waits = list(si.on_wait) if si is not None else []
    if len(waits) > 1:
        si.on_wait = waits[:1]
        for w in waits[1:]:
            extra = nc.sync.drain()
            esi = extra.ins.sync_info
            if esi is None:
                extra.ins.sync_info = mybir.SyncInfo(on_wait=[w], on_update=[])
            else:
                esi.on_wait = [w]
    nc.all_engine_barrier()
    assert self.sems is not None
    popped = nc._tile_sem_poison_stack.pop()
    assert popped is self._sem_poison
    nc.clear_and_free_semaphores(list(self.sems.allocated().values()))
    nc.all_engine_barrier()


tile.TileContext._drain_and_barrier = _patched_drain_and_barrier


def _split_multi_waits(nc):
    """Walrus in this container supports a single sem wait per instruction.
    Move extra waits onto dedicated no-op instructions on the same engine,
    inserted immediately before (engine program order preserves semantics)."""
    n_split = 0
    for fn in nc.m.functions:
        for bb in fn.blocks:
            out = []
            for ins in bb.instructions:
                si = ins.sync_info
                waits = list(si.on_wait) if si is not None else []
                if len(waits) > 1:
                    si.on_wait = [waits[-1]]
                    for i, w in enumerate(waits[:-1]):
                        nop = mybir.InstNoOp(
                            name=f"{ins.name}-sw{i}",
                            engine=ins.engine,
                            bass_nofuse=True,
                            sync_info=mybir.SyncInfo(on_wait=[w], on_update=[]),
                        )
                        out.append(nop)
                        n_split += 1
                out.append(ins)
            bb.instructions[:] = out
    return n_split


def _core_plan(k):
    g = k % 4
    return k // 4, g, g  # batch, head-group, token-quarter


def _tri_mask():
    """{0,1}[kv 128, q 128] within-tile causal keep (kv <= q)."""
    a = np.arange(P)[:, None]
    qq = np.arange(P)[None, :]
    return (a <= qq).astype(np.float32).astype(BF16)


def _build_nc(reps=1, ablate=(), debug=False):
    ablate = set(ablate)
    nc = bass.Bass()

    def din(name, shape, dt=BF):
        return nc.declare_dram_parameter(name, list(shape), dt, isOutput=False)

    xT_d = din("xT", (P, 8, S))
    mask_d = din("mask", (P, P))
    wq_d = din("wq", (P, 8, 256))
    wk_d = din("wk", (P, 8, 256))
    wv_d = din("wv", (P, 8, 256))
    wo_d = din("wo", (P, 8, D))
    w1_d = din("w1", (P, 8, FF))
    w2_d = din("w2", (P, 32, D))
    bqT_d = din("bqT", (P, 2), FP32)
    bkT_d = din("bkT", (P, 2), FP32)
    bvT_d = din("bvT", (P, 2), FP32)
    boT_d = din("boT", (P, 8), FP32)
    b1T_d = din("b1T", (P, 32), FP32)
    rows_d = din("rows", (3, D), FP32)  # b2 / gamma / beta
    eye_d = din("eye", (P, P), BF)
    sel_d = din("sel", (P, 8), FP32)
    out_d = nc.declare_dram_parameter("out", [512, D], FP32, isOutput=True)
    attv_dbg_d = (
        nc.declare_dram_parameter("attV_dbg", [P, 2, S], BF, isOutput=True)
        if debug
        else None
    )
    # one half-size AllToAll per local head pair (128-row chunks per dest;
    # cross-batch chunks are sel-zeroed and cancel in the recombine add)
    a2a_in = [nc.dram_tensor(f"a2a{i}in", [1024, 512], BF) for i in range(2)]
    a2a_out = [nc.dram_tensor(f"a2a{i}out", [1024, 512], BF) for i in range(2)]
    GROUPS = [[0, 1, 2, 3, 4, 5, 6, 7]]

    with ExitStack() as top:
        tc = top.enter_context(tile.TileContext(nc))

        const = top.enter_context(tc.tile_pool(name="const", bufs=1))
        persist = top.enter_context(tc.tile_pool(name="persist", bufs=1))

        # ---- constants (tiles only; DMAs are emitted after the critical
        # wq/wk/xT loads so they don't head-of-line delay the first matmul) --
        ones_sb = const.tile([P, P], FP32, tag="ones")
        nc.vector.memset(ones_sb[:], 1.0)
        eye_sb = const.tile([P, P], BF, tag="eye")
        bq_sb = const.tile([P, 2], FP32, tag="bq")
        bk_sb = const.tile([P, 2], FP32, tag="bk")
        bv_sb = const.tile([P, 2], FP32, tag="bv")
        sel_sb = const.tile([P, 8], FP32, tag="sel")
        bo_sb = const.tile([P, 8], FP32, tag="bo")
        b1_sb = const.tile([P, 32], FP32, tag="b1")
        mask_sb = const.tile([P, P], BF, tag="mask")
        # b2 at row 0, gamma at row 32, beta at row 64 (matmul rhs base part.)
        rows_sb = const.tile([P, D], FP32, tag="rows")

        def _const_dmas():
            nc.sync.dma_start(bk_sb[:], bkT_d[:])
            nc.sync.dma_start(bq_sb[:], bqT_d[:])
            nc.sync.dma_start(bv_sb[:], bvT_d[:])
            nc.sync.dma_start(mask_sb[:], mask_d[:])
            nc.sync.dma_start(sel_sb[:], sel_d[:])
            nc.sync.dma_start(eye_sb[:], eye_d[:])
            nc.sync.dma_start(bo_sb[:], boT_d[:])
            nc.sync.dma_start(b1_sb[:], b1T_d[:])
            nc.sync.dma_start(rows_sb[0:1, :], rows_d[0:1, :])
            nc.sync.dma_start(rows_sb[32:33, :], rows_d[1:2, :])
            nc.sync.dma_start(rows_sb[64:65, :], rows_d[2:3, :])

        attV = persist.tile([P, 2, S], BF, tag="attV")      # local heads
        attVf = persist.tile([P, 8, 512], BF, tag="attVf")  # post-A2A full
        attnTb = persist.tile([P, 8, 512], BF, tag="attnTb")
        # Wo weights prefetched during phase 1 (needed right after the A2A);
        # the DMA is issued after the xT/wqkv loads so it doesn't delay them
        wo_sb = persist.tile([P, 8, D], BF, tag="wo")
        # Wo kc0-3 partial (computed during head-pair-1 attention, bo folded)
        woPart = persist.tile([P, 8, 512], FP32, tag="woPart")
        if ablate:
            nc.vector.memset(attV[:], 0.25)
            nc.vector.memset(attVf[:], 0.25)
            nc.vector.memset(attnTb[:], 0.25)
            nc.vector.memset(woPart[:], 0.25)

        def _one_iter():
            _skip_proj = "compute" in ablate
            _skip_ffn = "ffn" in ablate or "compute" in ablate
            _n_hc = 0 if ("attn" in ablate or "compute" in ablate) else 2

            with ExitStack() as phx:
                # xp holds xT during projections; the slot is then reused
                # for the first half of W1 (same shape) — the prefetch DMA
                # carries a WAR dep on the last xT reader.
                xp = phx.enter_context(tc.tile_pool(name="xp", bufs=1))

                # ========= phase 1: projections interleaved with attention ==
                with ExitStack() as ph1:
                    p1 = ph1.enter_context(tc.tile_pool(name="p1", bufs=1))
                    wpool = ph1.enter_context(tc.tile_pool(name="wqkv", bufs=3))
                    ptp = ph1.enter_context(tc.tile_pool(name="ptp", bufs=20))
                    srec = ph1.enter_context(tc.tile_pool(name="srec", bufs=2))
                    stg = ph1.enter_context(tc.tile_pool(name="stg", bufs=3))
                    pp_o = ph1.enter_context(
                        tc.tile_pool(name="ppo", bufs=2, space="PSUM")
                    )

                    pps = ph1.enter_context(
                        tc.tile_pool(name="pps", bufs=2, space="PSUM")
                    )

                    KT = p1.tile([P, 2, S], BF, tag="KT")
                    Vp = p1.tile([P, 16, 4 * 65], BF, tag="Vp")
                    QT = p1.tile([P, 2, S], BF, tag="QT")
                    # ones in the 65th column of every (t, head) V block: one
                    # big memset; V copies below overwrite only cols 0..63.
                    nc.vector.memset(Vp[:], 1.0)

                    # weights first (small, needed immediately), then x in
                    # quarters so projections start after the first 1/4 lands
                    wk_sb = wpool.tile([P, 8, 256], BF, tag="w")
                    nc.sync.dma_start(wk_sb[:], wk_d[:])
                    xT_sb = xp.tile([P, 8, S], BF, tag="xp")
                    nc.sync.dma_start(xT_sb[:, :, 0:512], xT_d[:, :, 0:512])
                    wq_sb = wpool.tile([P, 8, 256], BF, tag="w")
                    nc.sync.dma_start(wq_sb[:], wq_d[:])
                    wv_sb = wpool.tile([P, 8, 256], BF, tag="w")
                    nc.sync.dma_start(wv_sb[:], wv_d[:])
                    for ng in range(1, 4):
                        nc.sync.dma_start(
                            xT_sb[:, :, ng * 512 : (ng + 1) * 512],
                            xT_d[:, :, ng * 512 : (ng + 1) * 512],
                        )
                    if reps == 1:
                        _const_dmas()
                    nc.sync.dma_start(wo_sb[:], wo_d[:])

                    def _proj_quarter(ng, pp_mm):
                        """Q/K/V projections for token quarter ng."""
                        if _skip_proj:
                            return
                        for dst, w_sb, b_sb in (
                            (KT, wk_sb, bk_sb),
                            (QT, wq_sb, bq_sb),
                        ):
                            for m in range(2):
                                ps = pp_mm.tile(
                                    [P, 512], FP32, tag="mm", name=f"qk_{m}_{ng}"
                                )
                                for kc in range(8):
                                    nc.tensor.matmul(
                                        ps[:],
                                        lhsT=w_sb[:, kc, m * P : (m + 1) * P],
                                        rhs=xT_sb[:, kc, ng * 512 : (ng + 1) * 512],
                                        start=(kc == 0),
                                        stop=(kc == 7),
                                    )
                                nc.vector.tensor_scalar_add(
                                    dst[:, m, ng * 512 : (ng + 1) * 512],
                                    ps[:],
                                    b_sb[:, m : m + 1],
                                )
                        # V for the same quarter (no bias)
                        for tt in range(4 * ng, 4 * ng + 4):
                            ps = pp_mm.tile([P, 512], FP32, tag="mm", name=f"v_{tt}")
                            for kc in range(8):
                                nc.tensor.matmul(
                                    ps[:, :256],
                                    lhsT=xT_sb[:, kc, tt * P : (tt + 1) * P],
                                    rhs=wv_sb[:, kc, :],
                                    start=(kc == 0),
                                    stop=(kc == 7),
                                )
                            dst = Vp[:, tt, :].rearrange("p (b j) -> p b j", j=65)[
                                :, :, 0:64
                            ]
                            nc.vector.tensor_copy(
                                dst, ps[:, :256].rearrange("p (b j) -> p b j", j=64)
                            )

                    pts_map = {}

                    def _attn_scores(hc, g4, t_lo, t_hi):
                        """Scores+exp+mask for kv chunks [t_lo, t_hi)."""
                        pts = pts_map.setdefault((hc, g4), [])
                        for t in range(t_lo, t_hi):
                            r = t - 4 * g4
                            qoff = max(r, 0) * P
                            s_pair = pps.tile(
                                [P, 1024], FP32, tag="s", name=f"s_{hc}_{g4}_{t}"
                            )
                            for i, hp in enumerate((0, 64)):
                                nc.tensor.matmul(
                                    s_pair[:, i * 512 + qoff : (i + 1) * 512],
                                    lhsT=KT[hp : hp + 64, hc, t * P : (t + 1) * P],
                                    rhs=QT[
                                        hp : hp + 64,
                                        hc,
                                        g4 * 512 + qoff : (g4 + 1) * 512,
                                    ],
                                    start=True,
                                    stop=True,
                                )
                            pt = ptp.tile(
                                [P, 1024], BF, tag="pt", name=f"pt_{hc}_{g4}_{t}"
                            )
                            pts.append(pt)
                            sv = s_pair[:].rearrange("p (h n) -> p h n", h=2)
                            pv = pt[:].rearrange("p (h n) -> p h n", h=2)
                            nc.scalar.activation(
                                pv[:, :, qoff:],
                                sv[:, :, qoff:],
                                AF.Exp,
                                scale=0.125,
                            )
                            if r >= 0:
                                nc.vector.tensor_tensor(
                                    pv[:, :, qoff : qoff + P],
                                    pv[:, :, qoff : qoff + P],
                                    mask_sb[:, None, :].to_broadcast([P, 2, P]),
                                    ALU.mult,
                                )

                    def _attn_av(hc, g4):
                        """AV+normalize+transpose+stage for a finished block."""
                        pts = pts_map.pop((hc, g4))
                        # av slots padded to a full PSUM bank so no two pools
                        # ever co-own a bank (interleaved accumulation groups
                        # sharing a bank corrupt all but the last region)
                        av = [
                            pp_o.tile(
                                [P, 4, 128],
                                FP32,
                                tag="av",
                                name=f"av_{hc}_{g4}_{i}",
                            )
                            for i in range(2)
                        ]
                        # AV region-major: PSUM accumulation groups sharing a
                        # bank must be sequential — interleaving them corrupts
                        # all but the last region (verified on hardware)
                        for i in range(2):
                            h = 2 * hc + i
                            for r4 in range(4):
                                tmax = 4 * g4 + r4
                                for t in range(tmax + 1):
                                    nc.tensor.matmul(
                                        av[i][:, r4, 0:65],
                                        lhsT=pts[t][
                                            :,
                                            i * 512 + r4 * P : i * 512 + (r4 + 1) * P,
                                        ],
                                        rhs=Vp[:, t, h * 65 : (h + 1) * 65],
                                        start=(t == 0),
                                        stop=(t == tmax),
                                    )
                        # normalize by the denominator in column 64, pack both
                        # heads q-major, transpose back to head-dim-major
                        avq = srec.tile(
                            [P, 4, P], BF, tag="avq", name=f"avq_{hc}_{g4}"
                        )
                        for i in range(2):
                            rec = srec.tile(
                                [P, 4], FP32, tag="rec", name=f"rc_{hc}_{g4}_{i}"
                            )
                            nc.vector.reciprocal(
                                rec[:],
                                av[i][:, :, 64:65].rearrange("p a b -> p (a b)"),
                            )
                            for r4 in range(4):
                                nc.vector.tensor_scalar_mul(
                                    avq[:, r4, i * 64 : (i + 1) * 64],
                                    av[i][:, r4, 0:64],
                                    rec[:, r4 : r4 + 1],
                                )
                        for r4 in range(4):
                            txp = pp_o.tile(
                                [P, P], BF, tag="av", name=f"tx_{hc}_{g4}_{r4}"
                            )
                            nc.tensor.transpose(txp[:], avq[:, r4, :], eye_sb[:])
                            qb = 4 * g4 + r4
                            nc.vector.tensor_scalar_add(
                                attV[:, hc, qb * P : (qb + 1) * P],
                                txp[:],
                                bv_sb[:, hc : hc + 1],
                            )
                        # stage this finished quarter for both dest batches
                        for j in (g4, g4 + 4):
                            st = stg.tile([P, 512], BF, tag="st", name=f"st{hc}_{j}")
                            nc.vector.tensor_scalar_mul(
                                st[:],
                                attV[:, hc, g4 * 512 : (g4 + 1) * 512],
                                sel_sb[:, j : j + 1],
                            )
                            nc.sync.dma_start(
                                a2a_in[hc][j * P : (j + 1) * P, :], st[:]
                            )

                    def _a2a(hc):
                        if reps > 1 or "a2a" in ablate:
                            # collectives cannot sit inside the timing repeat
                            # loop (and SWDGE DMAs don't compile there):
                            # substitute an equal-size local DMA on the SP
                            # ring (an ACT-ring transfer would stall the exp
                            # stream — hwdge transfers occupy their engine).
                            nc.sync.dma_start(a2a_out[hc][:], a2a_in[hc][:])
                        else:
                            nc.gpsimd.collective_compute(
                                "AllToAll",
                                ALU.bypass,
                                ins=[a2a_in[hc][:]],
                                outs=[a2a_out[hc][:]],
                                replica_groups=GROUPS,
                            )

                    def _gp_dma(out, in_):
                        # SWDGE DMA in the real build (rides behind the
                        # collective, leaves hwdge rings free); SP-ring in
                        # the For_i timing build where SWDGE can't compile.
                        if reps > 1:
                            nc.sync.dma_start(out, in_)
                        else:
                            nc.gpsimd.dma_start(out, in_)

                    # ---- flat pipeline: the first 4 score/exp units of each
                    # block are emitted BEFORE the previous block's AV pass,
                    # so the scalar engine always has exp work queued while
                    # the PE runs AV/projections. ----
                    LAG = 4
                    w1a_sb = xp.tile([P, 8, S], BF, tag="xp", name="w1a")

                    with ExitStack() as phA:
                        pp_proj = phA.enter_context(
                            tc.tile_pool(name="ppproj", bufs=2, space="PSUM")
                        )
                        _proj_quarter(0, pp_proj)
                        _proj_quarter(1, pp_proj)
                        if _n_hc:
                            _attn_scores(0, 0, 0, 4)
                        _proj_quarter(2, pp_proj)
                        if _n_hc:
                            _attn_scores(0, 1, 0, LAG)
                            _attn_av(0, 0)
                            _attn_scores(0, 1, LAG, 8)
                        _proj_quarter(3, pp_proj)

                    # proj psum released; wop (Wo kc0-3 partial) takes its
                    # banks for the rest of phase 1
                    with ExitStack() as phB:
                        wop = phB.enter_context(
                            tc.tile_pool(name="wop", bufs=2, space="PSUM")
                        )
                        if _n_hc:
                            _attn_scores(0, 2, 0, LAG)
                            _attn_av(0, 1)
                        nc.sync.dma_start(
                            w1a_sb[:, :, 0:1024], w1_d[:, :, 0:1024]
                        )
                        if _n_hc:
                            _attn_scores(0, 2, LAG, 12)
                            _attn_scores(0, 3, 0, LAG)
                            _attn_av(0, 2)
                        nc.sync.dma_start(
                            w1a_sb[:, :, 1024:2048], w1_d[:, :, 1024:2048]
                        )
                        if _n_hc:
                            _attn_scores(0, 3, LAG, 16)
                            _attn_scores(1, 0, 0, 4)
                            _attn_av(0, 3)
                            _a2a(0)
                            _attn_scores(1, 1, 0, LAG)
                            _attn_av(1, 0)
                            _attn_scores(1, 1, LAG, 8)
                            # readback A2A#1 (gpsimd ring in the real build:
                            # ordered after the collective, doesn't block SP
                            # staging)
                            hB = stg.tile(
                                [P, 4, 512], BF, tag="hB", name="hB", bufs=1
                            )
                            _gp_dma(
                                attVf[:, 0:4, :],
                                a2a_out[0][0:512, :].rearrange(
                                    "(i p) q -> p i q", p=P
                                ),
                            )
                            _gp_dma(
                                hB[:],
                                a2a_out[0][512:1024, :].rearrange(
                                    "(i p) q -> p i q", p=P
                                ),
                            )
                            _attn_scores(1, 2, 0, LAG)
                            _attn_av(1, 1)
                            _attn_scores(1, 2, LAG, 12)
                            nc.gpsimd.tensor_tensor(
                                attVf[:, 0:4, :], attVf[:, 0:4, :], hB[:], ALU.add
                            )
                            _attn_scores(1, 3, 0, LAG)
                            _attn_av(1, 2)
                            # Wo kc0-3 on head pair 0's exchanged halves,
                            # parked in SBUF with bo folded in
                            if not _skip_ffn:
                                for m in range(8):
                                    ps = wop.tile(
                                        [P, 512], FP32, tag="wo", name=f"wo1_{m}"
                                    )
                                    for kc in range(4):
                                        nc.tensor.matmul(
                                            ps[:],
                                            lhsT=wo_sb[:, kc, m * P : (m + 1) * P],
                                            rhs=attVf[:, kc, :],
                                            start=(kc == 0),
                                            stop=(kc == 3),
                                        )
                                    nc.vector.tensor_scalar_add(
                                        woPart[:, m, :], ps[:], bo_sb[:, m : m + 1]
                                    )
                            _attn_scores(1, 3, LAG, 16)
                            _attn_av(1, 3)
                            _a2a(1)
                    if attv_dbg_d is not None:
                        nc.sync.dma_start(attv_dbg_d[:], attV[:])

                # =========================== phase 2 ========================
                with ExitStack() as ph2:
                    p2 = ph2.enter_context(tc.tile_pool(name="p2", bufs=1))
                    # one streaming pool serves W1 chunks 4-7 first, then the
                    # W2 quarters reuse its slots (same shape)
                    wsp = ph2.enter_context(tc.tile_pool(name="wsp", bufs=4))
                    lnp = ph2.enter_context(tc.tile_pool(name="lnp", bufs=2))
                    smal = ph2.enter_context(tc.tile_pool(name="smal", bufs=2))
                    pp_mm = ph2.enter_context(
                        tc.tile_pool(name="ppmm2", bufs=8, space="PSUM")
                    )

                    def _w2_quarter(ng, qf):
                        w2t = wsp.tile(
                            [P, 8, 512], BF, tag="w", name=f"w2_{ng}_{qf}"
                        )
                        nc.sync.dma_start(
                            w2t[:],
                            w2_d[:, qf * 8 : (qf + 1) * 8, ng * 512 : (ng + 1) * 512],
                        )
                        return w2t

                    # ---- broadcast rows b2/gamma/beta -> [128, 1024] fp32 ----
                    b2b = p2.tile([P, D], FP32, tag="b2b")
                    gb = p2.tile([P, D], FP32, tag="gb")
                    bb = p2.tile([P, D], FP32, tag="bb")
                    for rp, dst in ((0, b2b), (32, gb), (64, bb)):
                        for hf in range(2):
                            psb = pp_mm.tile(
                                [P, 512], FP32, tag="mm", name=f"bc_{rp}_{hf}"
                            )
                            nc.tensor.matmul(
                                psb[:],
                                lhsT=ones_sb[rp : rp + 1, :],
                                rhs=rows_sb[rp : rp + 1, hf * 512 : (hf + 1) * 512],
                                start=True,
                                stop=True,
                            )
                            nc.vector.tensor_copy(
                                dst[:, hf * 512 : (hf + 1) * 512], psb[:]
                            )

                    # ---- readback + recombine A2A#2 (head pair 1), gpsimd
                    # ring so the waits ride behind the collective ----
                    hB2 = p2.tile([P, 4, 512], BF, tag="hB2")
                    _gp_dma(
                        attVf[:, 4:8, :],
                        a2a_out[1][0:512, :].rearrange("(i p) q -> p i q", p=P),
                    )
                    _gp_dma(
                        hB2[:],
                        a2a_out[1][512:1024, :].rearrange("(i p) q -> p i q", p=P),
                    )
                    nc.gpsimd.tensor_tensor(
                        attVf[:, 4:8, :], attVf[:, 4:8, :], hB2[:], ALU.add
                    )

                    # ---- Wo second half (head pair 1's kc4-7) + the parked
                    # kc0-3 partial -> attnTb ----
                    for m in range(8 if not _skip_ffn else 0):
                        ps = pp_mm.tile([P, 512], FP32, tag="mm", name=f"wo_{m}")
                        for kc in range(4, 8):
                            nc.tensor.matmul(
                                ps[:],
                                lhsT=wo_sb[:, kc, m * P : (m + 1) * P],
                                rhs=attVf[:, kc, :],
                                start=(kc == 4),
                                stop=(kc == 7),
                            )
                        nc.vector.tensor_tensor(
                            attnTb[:, m, :], ps[:], woPart[:, m, :], ALU.add
                        )

                    # ---- W1 + exact GELU (+b1): hT[f, q] bf16.  The resident
                    # w1a chunks 0-3 COMPUTE first (no DMA wait behind the
                    # collective), while chunks 4-7 stream in; their DMAs are
                    # issued upfront so they overlap the fg0-3 matmuls. ----
                    hT = p2.tile([P, 32, 512], BF, tag="hT")
                    w1ts = {}
                    for fg in (4, 5, 6, 7):
                        w1ts[fg] = wsp.tile(
                            [P, 8, 512], BF, tag="w", name=f"w1_{fg}"
                        )
                        nc.sync.dma_start(
                            w1ts[fg][:], w1_d[:, :, fg * 512 : (fg + 1) * 512]
                        )
                    for fg in range(8):
                        if fg < 4:
                            def _w1l(kc, fs, fg=fg):
                                base = fg * 512 + fs * P
                                return w1a_sb[:, kc, base : base + P]
                        else:
                            def _w1l(kc, fs, w1t=w1ts[fg]):
                                return w1t[:, kc, fs * P : (fs + 1) * P]

                        for fs in range(4 if not _skip_ffn else 0):
                            f = fg * 4 + fs
                            ps = pp_mm.tile([P, 512], FP32, tag="mm")
                            for kc in range(8):
                                nc.tensor.matmul(
                                    ps[:],
                                    lhsT=_w1l(kc, fs),
                                    rhs=attnTb[:, kc, :],
                                    start=(kc == 0),
                                    stop=(kc == 7),
                                )
                            nc.scalar.activation(
                                hT[:, f, :],
                                ps[:],
                                AF.Gelu,
                                bias=b1_sb[:, f : f + 1],
                                scale=1.0,
                            )

                    # ---- transpose attnTb -> attn_sb[q, dout] fp32 (+b2) ----
                    attn_sb = p2.tile([P, 4, D], FP32, tag="attn")
                    for m in range(8 if not _skip_ffn else 0):
                        for t4 in range(4):
                            pst = pp_mm.tile(
                                [P, 512], BF, tag="mm", name=f"tx_{m}_{t4}"
                            )
                            nc.tensor.transpose(
                                pst[:, 0:P],
                                attnTb[:, m, t4 * P : (t4 + 1) * P],
                                eye_sb[:],
                            )
                            nc.vector.tensor_tensor(
                                attn_sb[:, t4, m * P : (m + 1) * P],
                                pst[:, 0:P],
                                b2b[:, m * P : (m + 1) * P],
                                ALU.add,
                            )

                    # ---- W2 + residual (in place) + LayerNorm + out,
                    # t4-major so the LayerNorms stagger with the chains ----
                    for ng in range(2 if not _skip_ffn else 0):
                        w2_q = [_w2_quarter(ng, qf) for qf in range(4)]
                        for t4 in range(4):
                            ps = pp_mm.tile(
                                [P, 512], FP32, tag="mm", name=f"w2ps_{ng}_{t4}"
                            )
                            for fc in range(32):
                                nc.tensor.matmul(
                                    ps[:],
                                    lhsT=hT[:, fc, t4 * P : (t4 + 1) * P],
                                    rhs=w2_q[fc // 8][:, fc % 8, :],
                                    start=(fc == 0),
                                    stop=(fc == 31),
                                )
                            nc.vector.tensor_tensor(
                                attn_sb[:, t4, ng * 512 : (ng + 1) * 512],
                                ps[:],
                                attn_sb[:, t4, ng * 512 : (ng + 1) * 512],
                                ALU.add,
                            )
                            if ng == 1:
                                # ---- LayerNorm + out DMA for this t4 ----
                                yv = attn_sb[:, t4, :]
                                s1 = smal.tile([P, 1], FP32, tag="s1")
                                nc.vector.reduce_sum(s1[:], yv, axis=AX.X)
                                sqo = lnp.tile([P, D], FP32, tag="sc")
                                s2 = smal.tile([P, 1], FP32, tag="s2")
                                nc.scalar.activation(
                                    sqo[:], yv, AF.Square, accum_out=s2[:]
                                )
                                negmean = smal.tile([P, 1], FP32, tag="nm")
                                nc.vector.tensor_scalar_mul(
                                    negmean[:], s1[:], -1.0 / D
                                )
                                mm2 = smal.tile([P, 1], FP32, tag="mm2")
                                nc.vector.tensor_tensor(
                                    mm2[:], negmean[:], negmean[:], ALU.mult
                                )
                                bap = smal.tile([P, 1], FP32, tag="bap")
                                nc.vector.tensor_scalar(
                                    bap[:], mm2[:], -1.0, 1e-6, ALU.mult, ALU.add
                                )
                                std = smal.tile([P, 1], FP32, tag="std")
                                nc.scalar.activation(
                                    std[:], s2[:], AF.Sqrt, bias=bap[:], scale=1.0 / D
                                )
                                rstd = smal.tile([P, 1], FP32, tag="rstd")
                                nc.vector.reciprocal(rstd[:], std[:])
                                t1 = lnp.tile([P, D], FP32, tag="sc")
                                nc.vector.tensor_scalar(
                                    t1[:], yv, negmean[:], rstd[:], ALU.add, ALU.mult
                                )
                                nc.vector.tensor_tensor(
                                    t1[:], t1[:], gb[:], ALU.mult
                                )
                                nc.vector.tensor_tensor(
                                    t1[:], t1[:], bb[:], ALU.add
                                )
                                nc.sync.dma_start(
                                    out_d[t4 * P : (t4 + 1) * P, :], t1[:]
                                )

        if reps > 1:
            # timing-only variant: repeat the whole body on-device so HW
            # time dominates host/tunnel dispatch overhead. The body is
            # emitted four times per For_i iteration: the loop inserts an
            # all-engine barrier per iteration, so unrolling quarters that
            # cost and lets 3 of 4 adjacent layer-body transitions overlap
            # (the next body's xT load / projections fill the collective-
            # wait gap at the phase transition).
            _const_dmas()
            assert reps % 4 == 0, "timing reps must be divisible by 4"
            with tc.For_i(0, reps // 4, 1):
                for _ in range(4):
                    _one_iter()
        else:
            _one_iter()

    _split_multi_waits(nc)
    return nc


_CACHE = {}


def _get_nc(reps=1, ablate=()):
    key = ("nc", reps, tuple(sorted(ablate)))
    if key not in _CACHE:
        _CACHE[key] = _build_nc(reps, ablate)
    return _CACHE[key]


def _prep_in_maps(x, mask, Wq, bq, Wk, bk, Wv, bv, Wo, bo, W1, b1, W2, b2, gamma, beta):
    x = np.asarray(x, np.float32)

    def chunkT(w, nch):
        return np.ascontiguousarray(
            np.asarray(w, np.float32).astype(BF16).reshape(nch, P, -1).transpose(1, 0, 2)
        )

    # attVf kc-chunk order after the per-head-pair A2A is (hc, group):
    # kc = hc*4 + g' holds heads (g'*4 + 2*hc, +1) = old contiguous chunk
    # index 2*g' + hc.  Reorder Wo's contraction chunks to match.
    wo_h = np.ascontiguousarray(chunkT(Wo, 8)[:, [0, 2, 4, 6, 1, 3, 5, 7], :])
    w1_h = chunkT(W1, 8)
    w2_h = chunkT(W2, 32)
    Wq = np.asarray(Wq, np.float32)
    Wk = np.asarray(Wk, np.float32)
    Wv = np.asarray(Wv, np.float32)

    def bT(b, nch):
        return np.ascontiguousarray(np.asarray(b, np.float32).reshape(nch, P).T)

    bo_h = bT(bo, 8)
    b1_h = bT(b1, 32)
    rows_h = np.ascontiguousarray(
        np.stack(
            [
                np.asarray(b2, np.float32),
                np.asarray(gamma, np.float32),
                np.asarray(beta, np.float32),
            ]
        )
    )
    eye_h = np.eye(P, dtype=np.float32).astype(BF16)
    mask_h = _tri_mask()
    bq = np.asarray(bq, np.float32)
    bk = np.asarray(bk, np.float32)
    bv = np.asarray(bv, np.float32)

    in_maps = []
    plans = []
    for k in range(NCORES):
        b, g, c = _core_plan(k)
        xb = x[b]
        xT_h = np.ascontiguousarray(
            xb.T.astype(BF16).reshape(8, P, S).transpose(1, 0, 2)
        )
        hs = slice(g * 256, (g + 1) * 256)
        sel_h = np.zeros((P, 8), np.float32)
        sel_h[:, b * 4 : (b + 1) * 4] = 1.0
        in_maps.append(
            {
                "xT": xT_h,
                "mask": mask_h,
                "sel": sel_h,
                "wq": chunkT(Wq[:, hs], 8),
                "wk": chunkT(Wk[:, hs], 8),
                "wv": chunkT(Wv[:, hs], 8),
                "wo": wo_h,
                "w1": w1_h,
                "w2": w2_h,
                "bqT": bT(bq[hs], 2),
                "bkT": bT(bk[hs], 2),
                "bvT": bT(bv[hs], 2),
                "boT": bo_h,
                "b1T": b1_h,
                "rows": rows_h,
                "eye": eye_h,
            }
        )
        plans.append((b, c))
    return in_maps, plans


def kernel(**inputs):
    in_maps, plans = _prep_in_maps(**inputs)
    nc = _get_nc()
    res = run_bass_kernel_spmd(nc, in_maps, core_ids=list(range(NCORES)))
    out = np.zeros((B, S, D), np.float32)
    for k in range(NCORES):
        b, c = plans[k]
        out[b, c * 512 : (c + 1) * 512] = res.results[k]["out"]
    return out


# revision 39
# speedup vs baseline: 1.0461x; 1.0461x over previous
"""Trainium2 Bass kernel for nn_DecoderLayer_68212670595779.

Head-sharded attention (4 heads/core x batch over 8 cores), exchanged with
two half-size 8-rank AllToAlls (one per local head pair, overlapped with
attention / split Wo), token-quarter-parallel FFN. Attention uses q-major
AV accumulation ([q, 64+denominator] PSUM tiles, region-major accumulation
groups — interleaved groups sharing a PSUM bank corrupt all but the last
region on this hardware) with per-partition-scalar softmax normalization
and a PE transpose back to head-dim-major before staging.

Scheduling (measured 329-347 us/layer across runs on the repeat-loop
harness, ~+/-9 us run noise, rel err 5.878e-3; baseline was ~345 us;
a x4 For_i unroll measured no better than x2: 343/352 vs 347/329):
- QKV projections interleave with head-pair-0 attention per token quarter,
  and each block's first LAG score/exp units are emitted before the
  previous block's AV pass, so the scalar engine always has exp work
  queued while the PE runs AV/projections.
- W1's first half is prefetched during attention into the SBUF slot that
  held xT (same shape; SP-ring chunk DMAs with a WAR dep on the last xT
  reader), removing the phase-1 -> phase-2 weight-load stall.  The
  RESIDENT chunks 0-3 compute first (chunk 4-7 DMAs queue behind the
  collective stub on the SP ring; issuing them upfront overlaps the
  fg0-3 matmuls and unblocks the next iteration's xT prefetch early).
- In the real build, A2A readbacks ride the gpsimd SWDGE ring (ordered
  behind the collective; waits don't block SP staging/weight DMAs);
  recombine adds run on gpsimd.  In For_i timing builds SWDGE DMAs don't
  compile, so readbacks and the collective-stub copy use the SP ring
  (an ACT-ring transfer would stall the exp stream: hwdge transfers
  occupy their issuing engine for the transfer duration, measured).
- The kc0-3 half of Wo runs inside the head-pair-1 attention window
  (spare PSUM banks, partial parked in SBUF with bo folded in); only the
  kc4-7 half remains on the post-A2A#2 critical path.
- Residual accumulates in place in attn_sb; per-t4 W2 chains keep the
  four LayerNorms staggered.

Known HW cost model (microbenchmarked): every stationary change costs
~66ns unhidden LDWEIGHTS (no FWL in this toolchain; --enable-ldw-opt is
incompatible with bass Ldweights).  Chained 65-col AV matmuls: ~78ns.
Same-stationary consecutive MMs skip the reload (~43ns) but the tile
scheduler interleaves other matmuls between them, so a t-major AV
restructure measured SLOWER end-to-end (369 us) — kept r4-major chains.
Col-tiled AV pairs (64-col stationary, N=512) hit full peak (207ns/pair)
but need a separate denominator pass + per-column normalize; net ~even.
"""
import sys

sys.path.insert(0, "/opt/trn_rl_repo")

import numpy as np
import ml_dtypes
from contextlib import ExitStack

import concourse.bass as bass
import concourse.mybir as mybir
import concourse.tile as tile
from concourse.vector_clock import ScopedClock
from concourse.bass_utils import run_bass_kernel_spmd

BF16 = ml_dtypes.bfloat16
FP32 = mybir.dt.float32
BF = mybir.dt.bfloat16
AF = mybir.ActivationFunctionType
ALU = mybir.AluOpType
AX = mybir.AxisListType

B, S, D, H, HD, FF, P = 2, 2048, 1024, 16, 64, 4096, 128
NCORES = 8


# ---------------------------------------------------------------------------
# Workaround: this walrus build allows only ONE semaphore wait on a CTRL
# (Drain) instruction; TileContext's final drain carries one wait per busy
# proc. Split the waits across a chain of drains on the same engine.
def _patched_drain_and_barrier(self, tick_clock, wait_clock):
    nc = self.nc
    drain_inst = nc.sync.drain()
    wait_clock.add_sem_waits(
        drain_inst.ins, ScopedClock({None: tick_clock.global_clock})
    )
    si = drain_inst.ins.sync_info
    waits = list(si.on_wait) if si is not None else []
    if len(waits) > 1:
        si.on_wait = waits[:1]
        for w in waits[1:]:
            extra = nc.sync.drain()
            esi = extra.ins.sync_info
            if esi is None:
                extra.ins.sync_info = mybir.SyncInfo(on_wait=[w], on_update=[])
            else:
                esi.on_wait = [w]
    nc.all_engine_barrier()
    assert self.sems is not None
    popped = nc._tile_sem_poison_stack.pop()
    assert popped is self._sem_poison
    nc.clear_and_free_semaphores(list(self.sems.allocated().values()))
    nc.all_engine_barrier()


tile.TileContext._drain_and_barrier = _patched_drain_and_barrier


def _split_multi_waits(nc):
    """Walrus in this container supports a single sem wait per instruction.
    Move extra waits onto dedicated no-op instructions on the same engine,
    inserted immediately before (engine program order preserves semantics)."""
    n_split = 0
    for fn in nc.m.functions:
        for bb in fn.blocks:
            out = []
            for ins in bb.instructions:
                si = ins.sync_info
                waits = list(si.on_wait) if si is not None else []
                if len(waits) > 1:
                    si.on_wait = [waits[-1]]
                    for i, w in enumerate(waits[:-1]):
                        nop = mybir.InstNoOp(
                            name=f"{ins.name}-sw{i}",
                            engine=ins.engine,
                            bass_nofuse=True,
                            sync_info=mybir.SyncInfo(on_wait=[w], on_update=[]),
                        )
                        out.append(nop)
                        n_split += 1
                out.append(ins)
            bb.instructions[:] = out
    return n_split


def _core_plan(k):
    g = k % 4
    return k // 4, g, g  # batch, head-group, token-quarter


def _tri_mask():
    """{0,1}[kv 128, q 128] within-tile causal keep (kv <= q)."""
    a = np.arange(P)[:, None]
    qq = np.arange(P)[None, :]
    return (a <= qq).astype(np.float32).astype(BF16)


def _build_nc(reps=1, ablate=(), debug=False):
    ablate = set(ablate)
    nc = bass.Bass()

    def din(name, shape, dt=BF):
        return nc.declare_dram_parameter(name, list(shape), dt, isOutput=False)

    xT_d = din("xT", (P, 8, S))
    mask_d = din("mask", (P, P))
    wq_d = din("wq", (P, 8, 256))
    wk_d = din("wk", (P, 8, 256))
    wv_d = din("wv", (P, 8, 256))
    wo_d = din("wo", (P, 8, D))
    w1_d = din("w1", (P, 8, FF))
    w2_d = din("w2", (P, 32, D))
    bqT_d = din("bqT", (P, 2), FP32)
    bkT_d = din("bkT", (P, 2), FP32)
    bvT_d = din("bvT", (P, 2), FP32)
    boT_d = din("boT", (P, 8), FP32)
    b1T_d = din("b1T", (P, 32), FP32)
    rows_d = din("rows", (3, D), FP32)  # b2 / gamma / beta
    eye_d = din("eye", (P, P), BF)
    sel_d = din("sel", (P, 8), FP32)
    out_d = nc.declare_dram_parameter("out", [512, D], FP32, isOutput=True)
    attv_dbg_d = (
        nc.declare_dram_parameter("attV_dbg", [P, 2, S], BF, isOutput=True)
        if debug
        else None
    )
    # one half-size AllToAll per local head pair (128-row chunks per dest;
    # cross-batch chunks are sel-zeroed and cancel in the recombine add)
    a2a_in = [nc.dram_tensor(f"a2a{i}in", [1024, 512], BF) for i in range(2)]
    a2a_out = [nc.dram_tensor(f"a2a{i}out", [1024, 512], BF) for i in range(2)]
    GROUPS = [[0, 1, 2, 3, 4, 5, 6, 7]]

    with ExitStack() as top:
        tc = top.enter_context(tile.TileContext(nc))

        const = top.enter_context(tc.tile_pool(name="const", bufs=1))
        persist = top.enter_context(tc.tile_pool(name="persist", bufs=1))

        # ---- constants (tiles only; DMAs are emitted after the critical
        # wq/wk/xT loads so they don't head-of-line delay the first matmul) --
        ones_sb = const.tile([P, P], FP32, tag="ones")
        nc.vector.memset(ones_sb[:], 1.0)
        eye_sb = const.tile([P, P], BF, tag="eye")
        bq_sb = const.tile([P, 2], FP32, tag="bq")
        bk_sb = const.tile([P, 2], FP32, tag="bk")
        bv_sb = const.tile([P, 2], FP32, tag="bv")
        sel_sb = const.tile([P, 8], FP32, tag="sel")
        bo_sb = const.tile([P, 8], FP32, tag="bo")
        b1_sb = const.tile([P, 32], FP32, tag="b1")
        mask_sb = const.tile([P, P], BF, tag="mask")
        # b2 at row 0, gamma at row 32, beta at row 64 (matmul rhs base part.)
        rows_sb = const.tile([P, D], FP32, tag="rows")

        def _const_dmas():
            nc.sync.dma_start(bk_sb[:], bkT_d[:])
            nc.sync.dma_start(bq_sb[:], bqT_d[:])
            nc.sync.dma_start(bv_sb[:], bvT_d[:])
            nc.sync.dma_start(mask_sb[:], mask_d[:])
            nc.sync.dma_start(sel_sb[:], sel_d[:])
            nc.sync.dma_start(eye_sb[:], eye_d[:])
            nc.sync.dma_start(bo_sb[:], boT_d[:])
            nc.sync.dma_start(b1_sb[:], b1T_d[:])
            nc.sync.dma_start(rows_sb[0:1, :], rows_d[0:1, :])
            nc.sync.dma_start(rows_sb[32:33, :], rows_d[1:2, :])
            nc.sync.dma_start(rows_sb[64:65, :], rows_d[2:3, :])

        attV = persist.tile([P, 2, S], BF, tag="attV")      # local heads
        attVf = persist.tile([P, 8, 512], BF, tag="attVf")  # post-A2A full
        attnTb = persist.tile([P, 8, 512], BF, tag="attnTb")
        # Wo weights prefetched during phase 1 (needed right after the A2A);
        # the DMA is issued after the xT/wqkv loads so it doesn't delay them
        wo_sb = persist.tile([P, 8, D], BF, tag="wo")
        # Wo kc0-3 partial (computed during head-pair-1 attention, bo folded)
        woPart = persist.tile([P, 8, 512], FP32, tag="woPart")
        if ablate:
            nc.vector.memset(attV[:], 0.25)
            nc.vector.memset(attVf[:], 0.25)
            nc.vector.memset(attnTb[:], 0.25)
            nc.vector.memset(woPart[:], 0.25)

        def _one_iter():
            _skip_proj = "compute" in ablate
            _skip_ffn = "ffn" in ablate or "compute" in ablate
            _n_hc = 0 if ("attn" in ablate or "compute" in ablate) else 2

            with ExitStack() as phx:
                # xp holds xT during projections; the slot is then reused
                # for the first half of W1 (same shape) — the prefetch DMA
                # carries a WAR dep on the last xT reader.
                xp = phx.enter_context(tc.tile_pool(name="xp", bufs=1))

                # ========= phase 1: projections interleaved with attention ==
                with ExitStack() as ph1:
                    p1 = ph1.enter_context(tc.tile_pool(name="p1", bufs=1))
                    wpool = ph1.enter_context(tc.tile_pool(name="wqkv", bufs=3))
                    ptp = ph1.enter_context(tc.tile_pool(name="ptp", bufs=20))
                    srec = ph1.enter_context(tc.tile_pool(name="srec", bufs=2))
                    stg = ph1.enter_context(tc.tile_pool(name="stg", bufs=3))
                    pp_o = ph1.enter_context(
                        tc.tile_pool(name="ppo", bufs=2, space="PSUM")
                    )

                    pps = ph1.enter_context(
                        tc.tile_pool(name="pps", bufs=2, space="PSUM")
                    )

                    KT = p1.tile([P, 2, S], BF, tag="KT")
                    Vp = p1.tile([P, 16, 4 * 65], BF, tag="Vp")
                    QT = p1.tile([P, 2, S], BF, tag="QT")
                    # ones in the 65th column of every (t, head) V block: one
                    # big memset; V copies below overwrite only cols 0..63.
                    nc.vector.memset(Vp[:], 1.0)

                    # weights first (small, needed immediately), then x in
                    # quarters so projections start after the first 1/4 lands
                    wk_sb = wpool.tile([P, 8, 256], BF, tag="w")
                    nc.sync.dma_start(wk_sb[:], wk_d[:])
                    xT_sb = xp.tile([P, 8, S], BF, tag="xp")
                    nc.sync.dma_start(xT_sb[:, :, 0:512], xT_d[:, :, 0:512])
                    wq_sb = wpool.tile([P, 8, 256], BF, tag="w")
                    nc.sync.dma_start(wq_sb[:], wq_d[:])
                    wv_sb = wpool.tile([P, 8, 256], BF, tag="w")
                    nc.sync.dma_start(wv_sb[:], wv_d[:])
                    for ng in range(1, 4):
                        nc.sync.dma_start(
                            xT_sb[:, :, ng * 512 : (ng + 1) * 512],
                            xT_d[:, :, ng * 512 : (ng + 1) * 512],
                        )
                    if reps == 1:
                        _const_dmas()
                    nc.sync.dma_start(wo_sb[:], wo_d[:])

                    def _proj_quarter(ng, pp_mm):
                        """Q/K/V projections for token quarter ng."""
                        if _skip_proj:
                            return
                        for dst, w_sb, b_sb in (
                            (KT, wk_sb, bk_sb),
                            (QT, wq_sb, bq_sb),
                        ):
                            for m in range(2):
                                ps = pp_mm.tile(
                                    [P, 512], FP32, tag="mm", name=f"qk_{m}_{ng}"
                                )
                                for kc in range(8):
                                    nc.tensor.matmul(
                                        ps[:],
                                        lhsT=w_sb[:, kc, m * P : (m + 1) * P],
                                        rhs=xT_sb[:, kc, ng * 512 : (ng + 1) * 512],
                                        start=(kc == 0),
                                        stop=(kc == 7),
                                    )
                                nc.vector.tensor_scalar_add(
                                    dst[:, m, ng * 512 : (ng + 1) * 512],
                                    ps[:],
                                    b_sb[:, m : m + 1],
                                )
                        # V for the same quarter (no bias)
                        for tt in range(4 * ng, 4 * ng + 4):
                            ps = pp_mm.tile([P, 512], FP32, tag="mm", name=f"v_{tt}")
                            for kc in range(8):
                                nc.tensor.matmul(
                                    ps[:, :256],
                                    lhsT=xT_sb[:, kc, tt * P : (tt + 1) * P],
                                    rhs=wv_sb[:, kc, :],
                                    start=(kc == 0),
                                    stop=(kc == 7),
                                )
                            dst = Vp[:, tt, :].rearrange("p (b j) -> p b j", j=65)[
                                :, :, 0:64
                            ]
                            nc.vector.tensor_copy(
                                dst, ps[:, :256].rearrange("p (b j) -> p b j", j=64)
                            )

                    pts_map = {}

                    def _attn_scores(hc, g4, t_lo, t_hi):
                        """Scores+exp+mask for kv chunks [t_lo, t_hi)."""
                        pts = pts_map.setdefault((hc, g4), [])
                        for t in range(t_lo, t_hi):
                            r = t - 4 * g4
                            qoff = max(r, 0) * P
                            s_pair = pps.tile(
                                [P, 1024], FP32, tag="s", name=f"s_{hc}_{g4}_{t}"
                            )
                            for i, hp in enumerate((0, 64)):
                                nc.tensor.matmul(
                                    s_pair[:, i * 512 + qoff : (i + 1) * 512],
                                    lhsT=KT[hp : hp + 64, hc, t * P : (t + 1) * P],
                                    rhs=QT[
                                        hp : hp + 64,
                                        hc,
                                        g4 * 512 + qoff : (g4 + 1) * 512,
                                    ],
                                    start=True,
                                    stop=True,
                                )
                            pt = ptp.tile(
                                [P, 1024], BF, tag="pt", name=f"pt_{hc}_{g4}_{t}"
                            )
                            pts.append(pt)
                            sv = s_pair[:].rearrange("p (h n) -> p h n", h=2)
                            pv = pt[:].rearrange("p (h n) -> p h n", h=2)
                            nc.scalar.activation(
                                pv[:, :, qoff:],
                                sv[:, :, qoff:],
                                AF.Exp,
                                scale=0.125,
                            )
                            if r >= 0:
                                nc.vector.tensor_tensor(
                                    pv[:, :, qoff : qoff + P],
                                    pv[:, :, qoff : qoff + P],
                                    mask_sb[:, None, :].to_broadcast([P, 2, P]),
                                    ALU.mult,
                                )

                    def _attn_av(hc, g4):
                        """AV+normalize+transpose+stage for a finished block."""
                        pts = pts_map.pop((hc, g4))
                        # av slots padded to a full PSUM bank so no two pools
                        # ever co-own a bank (interleaved accumulation groups
                        # sharing a bank corrupt all but the last region)
                        av = [
                            pp_o.tile(
                                [P, 4, 128],
                                FP32,
                                tag="av",
                                name=f"av_{hc}_{g4}_{i}",
                            )
                            for i in range(2)
                        ]
                        # AV region-major: PSUM accumulation groups sharing a
                        # bank must be sequential — interleaving them corrupts
                        # all but the last region (verified on hardware)
                        for i in range(2):
                            h = 2 * hc + i
                            for r4 in range(4):
                                tmax = 4 * g4 + r4
                                for t in range(tmax + 1):
                                    nc.tensor.matmul(
                                        av[i][:, r4, 0:65],
                                        lhsT=pts[t][
                                            :,
                                            i * 512 + r4 * P : i * 512 + (r4 + 1) * P,
                                        ],
                                        rhs=Vp[:, t, h * 65 : (h + 1) * 65],
                                        start=(t == 0),
                                        stop=(t == tmax),
                                    )
                        # normalize by the denominator in column 64, pack both
                        # heads q-major, transpose back to head-dim-major
                        avq = srec.tile(
                            [P, 4, P], BF, tag="avq", name=f"avq_{hc}_{g4}"
                        )
                        for i in range(2):
                            rec = srec.tile(
                                [P, 4], FP32, tag="rec", name=f"rc_{hc}_{g4}_{i}"
                            )
                            nc.vector.reciprocal(
                                rec[:],
                                av[i][:, :, 64:65].rearrange("p a b -> p (a b)"),
                            )
                            for r4 in range(4):
                                nc.vector.tensor_scalar_mul(
                                    avq[:, r4, i * 64 : (i + 1) * 64],
                                    av[i][:, r4, 0:64],
                                    rec[:, r4 : r4 + 1],
                                )
                        for r4 in range(4):
                            txp = pp_o.tile(
                                [P, P], BF, tag="av", name=f"tx_{hc}_{g4}_{r4}"
                            )
                            nc.tensor.transpose(txp[:], avq[:, r4, :], eye_sb[:])
                            qb = 4 * g4 + r4
                            nc.vector.tensor_scalar_add(
                                attV[:, hc, qb * P : (qb + 1) * P],
                                txp[:],
                                bv_sb[:, hc : hc + 1],
                            )
                        # stage this finished quarter for both dest batches
                        for j in (g4, g4 + 4):
                            st = stg.tile([P, 512], BF, tag="st", name=f"st{hc}_{j}")
                            nc.vector.tensor_scalar_mul(
                                st[:],
                                attV[:, hc, g4 * 512 : (g4 + 1) * 512],
                                sel_sb[:, j : j + 1],
                            )
                            nc.sync.dma_start(
                                a2a_in[hc][j * P : (j + 1) * P, :], st[:]
                            )

                    def _a2a(hc):
                        if reps > 1 or "a2a" in ablate:
                            # collectives cannot sit inside the timing repeat
                            # loop (and SWDGE DMAs don't compile there):
                            # substitute an equal-size local DMA on the SP
                            # ring (an ACT-ring transfer would stall the exp
                            # stream — hwdge transfers occupy their engine).
                            nc.sync.dma_start(a2a_out[hc][:], a2a_in[hc][:])
                        else:
                            nc.gpsimd.collective_compute(
                                "AllToAll",
                                ALU.bypass,
                                ins=[a2a_in[hc][:]],
                                outs=[a2a_out[hc][:]],
                                replica_groups=GROUPS,
                            )

                    def _gp_dma(out, in_):
                        # SWDGE DMA in the real build (rides behind the
                        # collective, leaves hwdge rings free); SP-ring in
                        # the For_i timing build where SWDGE can't compile.
                        if reps > 1:
                            nc.sync.dma_start(out, in_)
                        else:
                            nc.gpsimd.dma_start(out, in_)

                    # ---- flat pipeline: the first 4 score/exp units of each
                    # block are emitted BEFORE the previous block's AV pass,
                    # so the scalar engine always has exp work queued while
                    # the PE runs AV/projections. ----
                    LAG = 4
                    w1a_sb = xp.tile([P, 8, S], BF, tag="xp", name="w1a")

                    with ExitStack() as phA:
                        pp_proj = phA.enter_context(
                            tc.tile_pool(name="ppproj", bufs=2, space="PSUM")
                        )
                        _proj_quarter(0, pp_proj)
                        _proj_quarter(1, pp_proj)
                        if _n_hc:
                            _attn_scores(0, 0, 0, 4)
                        _proj_quarter(2, pp_proj)
                        if _n_hc:
                            _attn_scores(0, 1, 0, LAG)
                            _attn_av(0, 0)
                            _attn_scores(0, 1, LAG, 8)
                        _proj_quarter(3, pp_proj)

                    # proj psum released; wop (Wo kc0-3 partial) takes its
                    # banks for the rest of phase 1
                    with ExitStack() as phB:
                        wop = phB.enter_context(
                            tc.tile_pool(name="wop", bufs=2, space="PSUM")
                        )
                        if _n_hc:
                            _attn_scores(0, 2, 0, LAG)
                            _attn_av(0, 1)
                        nc.sync.dma_start(
                            w1a_sb[:, :, 0:1024], w1_d[:, :, 0:1024]
                        )
                        if _n_hc:
                            _attn_scores(0, 2, LAG, 12)
                            _attn_scores(0, 3, 0, LAG)
                            _attn_av(0, 2)
                        nc.sync.dma_start(
                            w1a_sb[:, :, 1024:2048], w1_d[:, :, 1024:2048]
                        )
                        if _n_hc:
                            _attn_scores(0, 3, LAG, 16)
                            _attn_scores(1, 0, 0, 4)
                            _attn_av(0, 3)
                            _a2a(0)
                            _attn_scores(1, 1, 0, LAG)
                            _attn_av(1, 0)
                            _attn_scores(1, 1, LAG, 8)
                            # readback A2A#1 (gpsimd ring in the real build:
                            # ordered after the collective, doesn't block SP
                            # staging)
                            hB = stg.tile(
                                [P, 4, 512], BF, tag="hB", name="hB", bufs=1
                            )
                            _gp_dma(
                                attVf[:, 0:4, :],
                                a2a_out[0][0:512, :].rearrange(
                                    "(i p) q -> p i q", p=P
                                ),
                            )
                            _gp_dma(
                                hB[:],
                                a2a_out[0][512:1024, :].rearrange(
                                    "(i p) q -> p i q", p=P
                                ),
                            )
                            _attn_scores(1, 2, 0, LAG)
                            _attn_av(1, 1)
                            _attn_scores(1, 2, LAG, 12)
                            nc.gpsimd.tensor_tensor(
                                attVf[:, 0:4, :], attVf[:, 0:4, :], hB[:], ALU.add
                            )
                            _attn_scores(1, 3, 0, LAG)
                            _attn_av(1, 2)
                            # Wo kc0-3 on head pair 0's exchanged halves,
                            # parked in SBUF with bo folded in
                            if not _skip_ffn:
                                for m in range(8):
                                    ps = wop.tile(
                                        [P, 512], FP32, tag="wo", name=f"wo1_{m}"
                                    )
                                    for kc in range(4):
                                        nc.tensor.matmul(
                                            ps[:],
                                            lhsT=wo_sb[:, kc, m * P : (m + 1) * P],
                                            rhs=attVf[:, kc, :],
                                            start=(kc == 0),
                                            stop=(kc == 3),
                                        )
                                    nc.vector.tensor_scalar_add(
                                        woPart[:, m, :], ps[:], bo_sb[:, m : m + 1]
                                    )
                            _attn_scores(1, 3, LAG, 16)
                            _attn_av(1, 3)
                            _a2a(1)
                    if attv_dbg_d is not None:
                        nc.sync.dma_start(attv_dbg_d[:], attV[:])

                # =========================== phase 2 ========================
                with ExitStack() as ph2:
                    p2 = ph2.enter_context(tc.tile_pool(name="p2", bufs=1))
                    # one streaming pool serves W1 chunks 4-7 first, then the
                    # W2 quarters reuse its slots (same shape)
                    wsp = ph2.enter_context(tc.tile_pool(name="wsp", bufs=4))
                    lnp = ph2.enter_context(tc.tile_pool(name="lnp", bufs=2))
                    smal = ph2.enter_context(tc.tile_pool(name="smal", bufs=2))
                    pp_mm = ph2.enter_context(
                        tc.tile_pool(name="ppmm2", bufs=8, space="PSUM")
                    )

                    def _w2_quarter(ng, qf):
                        w2t = wsp.tile(
                            [P, 8, 512], BF, tag="w", name=f"w2_{ng}_{qf}"
                        )
                        nc.sync.dma_start(
                            w2t[:],
                            w2_d[:, qf * 8 : (qf + 1) * 8, ng * 512 : (ng + 1) * 512],
                        )
                        return w2t

                    # ---- broadcast rows b2/gamma/beta -> [128, 1024] fp32 ----
                    b2b = p2.tile([P, D], FP32, tag="b2b")
                    gb = p2.tile([P, D], FP32, tag="gb")
                    bb = p2.tile([P, D], FP32, tag="bb")
                    for rp, dst in ((0, b2b), (32, gb), (64, bb)):
                        for hf in range(2):
                            psb = pp_mm.tile(
                                [P, 512], FP32, tag="mm", name=f"bc_{rp}_{hf}"
                            )
                            nc.tensor.matmul(
                                psb[:],
                                lhsT=ones_sb[rp : rp + 1, :],
                                rhs=rows_sb[rp : rp + 1, hf * 512 : (hf + 1) * 512],
                                start=True,
                                stop=True,
                            )
                            nc.vector.tensor_copy(
                                dst[:, hf * 512 : (hf + 1) * 512], psb[:]
                            )

                    # ---- readback + recombine A2A#2 (head pair 1), gpsimd
                    # ring so the waits ride behind the collective ----
                    hB2 = p2.tile([P, 4, 512], BF, tag="hB2")
                    _gp_dma(
                        attVf[:, 4:8, :],
                        a2a_out[1][0:512, :].rearrange("(i p) q -> p i q", p=P),
                    )
                    _gp_dma(
                        hB2[:],
                        a2a_out[1][512:1024, :].rearrange("(i p) q -> p i q", p=P),
                    )
                    nc.gpsimd.tensor_tensor(
                        attVf[:, 4:8, :], attVf[:, 4:8, :], hB2[:], ALU.add
                    )

                    # ---- Wo second half (head pair 1's kc4-7) + the parked
                    # kc0-3 partial -> attnTb ----
                    for m in range(8 if not _skip_ffn else 0):
                        ps = pp_mm.tile([P, 512], FP32, tag="mm", name=f"wo_{m}")
                        for kc in range(4, 8):
                            nc.tensor.matmul(
                                ps[:],
                                lhsT=wo_sb[:, kc, m * P : (m + 1) * P],
                                rhs=attVf[:, kc, :],
                                start=(kc == 4),
                                stop=(kc == 7),
                            )
                        nc.vector.tensor_tensor(
                            attnTb[:, m, :], ps[:], woPart[:, m, :], ALU.add
                        )

                    # ---- W1 + exact GELU (+b1): hT[f, q] bf16.  The resident
                    # w1a chunks 0-3 COMPUTE first (no DMA wait behind the
                    # collective), while chunks 4-7 stream in; their DMAs are
                    # issued upfront so they overlap the fg0-3 matmuls. ----
                    hT = p2.tile([P, 32, 512], BF, tag="hT")
                    w1ts = {}
                    for fg in (4, 5, 6, 7):
                        w1ts[fg] = wsp.tile(
                            [P, 8, 512], BF, tag="w", name=f"w1_{fg}"
                        )
                        nc.sync.dma_start(
                            w1ts[fg][:], w1_d[:, :, fg * 512 : (fg + 1) * 512]
                        )
                    for fg in range(8):
                        if fg < 4:
                            def _w1l(kc, fs, fg=fg):
                                base = fg * 512 + fs * P
                                return w1a_sb[:, kc, base : base + P]
                        else:
                            def _w1l(kc, fs, w1t=w1ts[fg]):
                                return w1t[:, kc, fs * P : (fs + 1) * P]

                        for fs in range(4 if not _skip_ffn else 0):
                            f = fg * 4 + fs
                            ps = pp_mm.tile([P, 512], FP32, tag="mm")
                            for kc in range(8):
                                nc.tensor.matmul(
                                    ps[:],
                                    lhsT=_w1l(kc, fs),
                                    rhs=attnTb[:, kc, :],
                                    start=(kc == 0),
                                    stop=(kc == 7),
                                )
                            nc.scalar.activation(
                                hT[:, f, :],
                                ps[:],
                                AF.Gelu,
                                bias=b1_sb[:, f : f + 1],
                                scale=1.0,
                            )

                    # ---- transpose attnTb -> attn_sb[q, dout] fp32 (+b2) ----
                    attn_sb = p2.tile([P, 4, D], FP32, tag="attn")
                    for m in range(8 if not _skip_ffn else 0):
                        for t4 in range(4):
                            pst = pp_mm.tile(
                                [P, 512], BF, tag="mm", name=f"tx_{m}_{t4}"
                            )
                            nc.tensor.transpose(
                                pst[:, 0:P],
                                attnTb[:, m, t4 * P : (t4 + 1) * P],
                                eye_sb[:],
                            )
                            nc.vector.tensor_tensor(
                                attn_sb[:, t4, m * P : (m + 1) * P],
                                pst[:, 0:P],
                                b2b[:, m * P : (m + 1) * P],
                                ALU.add,
                            )

                    # ---- W2 + residual (in place) + LayerNorm + out,
                    # t4-major so the LayerNorms stagger with the chains ----
                    for ng in range(2 if not _skip_ffn else 0):
                        w2_q = [_w2_quarter(ng, qf) for qf in range(4)]
                        for t4 in range(4):
                            ps = pp_mm.tile(
                                [P, 512], FP32, tag="mm", name=f"w2ps_{ng}_{t4}"
                            )
                            for fc in range(32):
                                nc.tensor.matmul(
                                    ps[:],
                                    lhsT=hT[:, fc, t4 * P : (t4 + 1) * P],
                                    rhs=w2_q[fc // 8][:, fc % 8, :],
                                    start=(fc == 0),
                                    stop=(fc == 31),
                                )
                            nc.vector.tensor_tensor(
                                attn_sb[:, t4, ng * 512 : (ng + 1) * 512],
                                ps[:],
                                attn_sb[:, t4, ng * 512 : (ng + 1) * 512],
                                ALU.add,
                            )
                            if ng == 1:
                                # ---- LayerNorm + out DMA for this t4 ----
                                yv = attn_sb[:, t4, :]
                                s1 = smal.tile([P, 1], FP32, tag="s1")
                                nc.vector.reduce_sum(s1[:], yv, axis=AX.X)
                                sqo = lnp.tile([P, D], FP32, tag="sc")
                                s2 = smal.tile([P, 1], FP32, tag="s2")
                                nc.scalar.activation(
                                    sqo[:], yv, AF.Square, accum_out=s2[:]
                                )
                                negmean = smal.tile([P, 1], FP32, tag="nm")
                                nc.vector.tensor_scalar_mul(
                                    negmean[:], s1[:], -1.0 / D
                                )
                                mm2 = smal.tile([P, 1], FP32, tag="mm2")
                                nc.vector.tensor_tensor(
                                    mm2[:], negmean[:], negmean[:], ALU.mult
                                )
                                bap = smal.tile([P, 1], FP32, tag="bap")
                                nc.vector.tensor_scalar(
                                    bap[:], mm2[:], -1.0, 1e-6, ALU.mult, ALU.add
                                )
                                std = smal.tile([P, 1], FP32, tag="std")
                                nc.scalar.activation(
                                    std[:], s2[:], AF.Sqrt, bias=bap[:], scale=1.0 / D
                                )
                                rstd = smal.tile([P, 1], FP32, tag="rstd")
                                nc.vector.reciprocal(rstd[:], std[:])
                                t1 = lnp.tile([P, D], FP32, tag="sc")
                                nc.vector.tensor_scalar(
                                    t1[:], yv, negmean[:], rstd[:], ALU.add, ALU.mult
                                )
                                nc.vector.tensor_tensor(
                                    t1[:], t1[:], gb[:], ALU.mult
                                )
                                nc.vector.tensor_tensor(
                                    t1[:], t1[:], bb[:], ALU.add
                                )
                                nc.sync.dma_start(
                                    out_d[t4 * P : (t4 + 1) * P, :], t1[:]
                                )

        if reps > 1:
            # timing-only variant: repeat the whole body on-device so HW
            # time dominates host/tunnel dispatch overhead. The body is
            # emitted twice per For_i iteration: the loop inserts an
            # all-engine barrier per iteration, so unrolling halves that
            # cost and lets adjacent layer bodies overlap.
            _const_dmas()
            assert reps % 2 == 0, "timing reps must be even"
            with tc.For_i(0, reps // 2, 1):
                _one_iter()
                _one_iter()
        else:
            _one_iter()

    _split_multi_waits(nc)
    return nc


_CACHE = {}


def _get_nc(reps=1, ablate=()):
    key = ("nc", reps, tuple(sorted(ablate)))
    if key not in _CACHE:
        _CACHE[key] = _build_nc(reps, ablate)
    return _CACHE[key]


def _prep_in_maps(x, mask, Wq, bq, Wk, bk, Wv, bv, Wo, bo, W1, b1, W2, b2, gamma, beta):
    x = np.asarray(x, np.float32)

    def chunkT(w, nch):
        return np.ascontiguousarray(
            np.asarray(w, np.float32).astype(BF16).reshape(nch, P, -1).transpose(1, 0, 2)
        )

    # attVf kc-chunk order after the per-head-pair A2A is (hc, group):
    # kc = hc*4 + g' holds heads (g'*4 + 2*hc, +1) = old contiguous chunk
    # index 2*g' + hc.  Reorder Wo's contraction chunks to match.
    wo_h = np.ascontiguousarray(chunkT(Wo, 8)[:, [0, 2, 4, 6, 1, 3, 5, 7], :])
    w1_h = chunkT(W1, 8)
    w2_h = chunkT(W2, 32)
    Wq = np.asarray(Wq, np.float32)
    Wk = np.asarray(Wk, np.float32)
    Wv = np.asarray(Wv, np.float32)

    def bT(b, nch):
        return np.ascontiguousarray(np.asarray(b, np.float32).reshape(nch, P).T)

    bo_h = bT(bo, 8)
    b1_h = bT(b1, 32)
    rows_h = np.ascontiguousarray(
        np.stack(
            [
                np.asarray(b2, np.float32),
                np.asarray(gamma, np.float32),
                np.asarray(beta, np.float32),
            ]
        )
    )
    eye_h = np.eye(P, dtype=np.float32).astype(BF16)
    mask_h = _tri_mask()
    bq = np.asarray(bq, np.float32)
    bk = np.asarray(bk, np.float32)
    bv = np.asarray(bv, np.float32)

    in_maps = []
    plans = []
    for k in range(NCORES):
        b, g, c = _core_plan(k)
        xb = x[b]
        xT_h = np.ascontiguousarray(
            xb.T.astype(BF16).reshape(8, P, S).transpose(1, 0, 2)
        )
        hs = slice(g * 256, (g + 1) * 256)
        sel_h = np.zeros((P, 8), np.float32)
        sel_h[:, b * 4 : (b + 1) * 4] = 1.0
        in_maps.append(
            {
                "xT": xT_h,
                "mask": mask_h,
                "sel": sel_h,
                "wq": chunkT(Wq[:, hs], 8),
                "wk": chunkT(Wk[:, hs], 8),
                "wv": chunkT(Wv[:, hs], 8),
                "wo": wo_h,
                "w1": w1_h,
                "w2": w2_h,
                "bqT": bT(bq[hs], 2),
                "bkT": bT(bk[hs], 2),
                "bvT": bT(bv[hs], 2),
                "boT": bo_h,
                "b1T": b1_h,
                "rows": rows_h,
                "eye": eye_h,
            }
        )
        plans.append((b, c))
    return in_maps, plans


def kernel(**inputs):
    in_maps, plans = _prep_in_maps(**inputs)
    nc = _get_nc()
    res = run_bass_kernel_spmd(nc, in_maps, core_ids=list(range(NCORES)))
    out = np.zeros((B, S, D), np.float32)
    for k in range(NCORES):
        b, c = plans[k]
        out[b, c * 512 : (c + 1) * 512] = res.results[k]["out"]
    return out
